# revision 18
# baseline (speedup 1.0000x reference)
"""GNN message-passing v4: batched dma_gather + tile-level selection matmuls.

vs v2: SEG=128 (one selection matrix per column covering the whole 128-dst
tile -> minimal padding), per-tile accumulator tiles (no shared-tile hazard
serialization), gather ring depth 3.
"""

import os
import sys

sys.path.insert(0, "/opt/trn_rl_repo")

import numpy as np
import ml_dtypes

BF16 = ml_dtypes.bfloat16

N_CORES = 8
P = 128
CALLCOLS = 64     # 128-edge columns per dma_gather call (num_idxs 8192, HW-validated)
TW = 128          # table row width (bf16 elements) -> 256B rows


def _make_cfg(n_nodes, n_edges, f_in=512, hid=64, n_cls=40):
    np_ = n_nodes // N_CORES
    assert np_ * N_CORES == n_nodes
    nw = (np_ + P - 1) // P
    npad = nw * P
    n_chunks = min(4, nw)
    tiles = [nw // n_chunks + (1 if i < nw % n_chunks else 0) for i in range(n_chunks)]
    tstart = np.concatenate([[0], np.cumsum(tiles)]).astype(int)
    cs = [int(tstart[i] * P) for i in range(n_chunks + 1)]
    cz = [cs[i + 1] - cs[i] for i in range(n_chunks)]
    crows = [1 + N_CORES * z for z in cz]
    cb = np.concatenate([[0], np.cumsum(crows)]).astype(int)
    assert max(crows) <= 32767
    return dict(
        N=n_nodes, E=n_edges, F=f_in, H=hid, C=n_cls, NP=np_, NW=nw, NPAD=npad,
        NCH=n_chunks, TILES=tiles, TSTART=tstart, CS=cs, CZ=cz,
        CROWS=crows, CB=cb, TOT=int(cb[-1]),
    )


FULL_CFG = _make_cfg(100000, 3200000)


# ---------------------------------------------------------------- host prep

def host_prep(cfg, x, edge_index, W1, b1, W2, b2):
    N, NP, NW, NCH = cfg["N"], cfg["NP"], cfg["NW"], cfg["NCH"]
    CS, CZ, CB = cfg["CS"], cfg["CZ"], cfg["CB"]
    src = np.asarray(edge_index[0]).astype(np.int64)
    dst = np.asarray(edge_index[1]).astype(np.int64)

    r = np.arange(N, dtype=np.int64) // NP
    l = np.arange(N, dtype=np.int64) % NP
    csb = np.asarray(CS)
    ch_of = np.searchsorted(csb, l, side="right") - 1
    czv = np.asarray(CZ + [1])[ch_of]
    locrow = 1 + r * czv + (l - csb[ch_of])

    # per-core sorted edge views + per-(chunk, tile) counts
    views = []
    cnts = np.zeros((N_CORES, NCH, NW), dtype=np.int64)
    for k in range(N_CORES):
        sel = (dst >= k * NP) & (dst < (k + 1) * NP)
        s_k = src[sel]
        d_k = dst[sel] - k * NP
        key = ch_of[s_k] * (NP + 1) + d_k
        order = np.argsort(key, kind="stable")
        s_k, d_k = s_k[order], d_k[order]
        ch_k = ch_of[s_k]
        lr_k = locrow[s_k]
        views.append((d_k, ch_k, lr_k))
        for c in range(NCH):
            m = ch_k == c
            tt = d_k[m] // P
            bc = np.bincount(tt, minlength=NW)
            cnts[k, c, :] = bc

    # common column layout: per (chunk, tile) max over cores of ceil(cnt/128)
    ncol_ct = np.maximum(1, (cnts + P - 1) // P).max(axis=0)  # [NCH, NW]
    # meta per chunk: list of (tile, start, stop, tc_last)
    common_meta = []
    for c in range(NCH):
        meta_c = []
        for t in range(NW):
            nc_ = int(ncol_ct[c, t])
            for j in range(nc_):
                meta_c.append([t, j == 0, j == nc_ - 1, j == nc_ - 1])
        common_meta.append(meta_c)
    totcols = int(ncol_ct.sum())

    # per-core index streams + selection matrices on the common layout
    idxp_all, s_all = [], []
    for k in range(N_CORES):
        d_k, ch_k, lr_k = views[k]
        idx_list, s_list = [], []
        for c in range(NCH):
            m = ch_k == c
            dc = d_k[m]
            lrc = lr_k[m]
            tt = dc // P
            # edges already sorted by dst within chunk -> tiles contiguous
            tstarts = np.concatenate([[0], np.cumsum(np.bincount(tt, minlength=NW))])
            for t in range(NW):
                a, b = int(tstarts[t]), int(tstarts[t + 1])
                cnt = b - a
                nc_ = int(ncol_ct[c, t])
                rows = np.zeros(nc_ * P, dtype=np.int16)
                rows[:cnt] = lrc[a:b]
                dloc = np.full(nc_ * P, -1, dtype=np.int64)
                dloc[:cnt] = dc[a:b] - t * P
                for j in range(nc_):
                    sm = np.zeros((P, P), dtype=BF16)
                    dj = dloc[j * P:(j + 1) * P]
                    val = dj >= 0
                    sm[np.arange(P)[val], dj[val]] = 1.0
                    s_list.append(sm)
                    idx_list.append(rows[j * P:(j + 1) * P])
        st = np.concatenate(idx_list)
        t16 = st.reshape(totcols * 8, 16).T
        idxp_all.append(np.ascontiguousarray(np.tile(t16, (8, 1))))
        s_all.append(np.ascontiguousarray(
            np.stack(s_list, axis=1).reshape(P, -1)))

    W1b = np.asarray(W1, dtype=np.float32).astype(BF16)
    W2b = np.asarray(W2, dtype=np.float32).astype(BF16)
    b1r = np.tile(np.asarray(b1, dtype=np.float32)[None, :], (P, 1))
    b2a = np.asarray(b2, dtype=np.float32) - np.asarray(W2, np.float32).sum(0)
    b2r = np.tile(b2a[None, :], (P, 1))
    in_maps = []
    xf = np.asarray(x, dtype=np.float32)
    for k in range(N_CORES):
        xT = np.ascontiguousarray(xf[k * NP:(k + 1) * NP].T).astype(BF16)
        in_maps.append(dict(
            xT=xT, W1=W1b, b1r=b1r, W2=W2b, b2r=b2r,
            idxp=idxp_all[k], smat=s_all[k],
        ))
    sched = dict(common_meta=common_meta, totcols=totcols)
    return sched, in_maps


# ---------------------------------------------------------------- device code

def build_program(cfg, sched):
    import concourse.bass as bass
    import concourse.bacc as bacc
    import concourse.mybir as mybir
    from concourse.tile import TileContext
    from concourse.masks import make_identity

    dt = mybir.dt
    N, F, H, C = cfg["N"], cfg["F"], cfg["H"], cfg["C"]
    NP, NW, NPAD, NCH = cfg["NP"], cfg["NW"], cfg["NPAD"], cfg["NCH"]
    CS, CZ, CB, CROWS = cfg["CS"], cfg["CZ"], cfg["CB"], cfg["CROWS"]
    TOT = cfg["TOT"]
    meta = sched["common_meta"]
    ncols_ch = [len(m) for m in meta]
    totcols = sched["totcols"]
    KF = F // P

    nc = bacc.Bacc(
        "TRN2", target_bir_lowering=False, debug=False, num_devices=N_CORES
    )
    xT = nc.declare_dram_parameter("xT", [F, NP], dt.bfloat16, isOutput=False)
    W1p = nc.declare_dram_parameter("W1", [F, H], dt.bfloat16, isOutput=False)
    b1p = nc.declare_dram_parameter("b1r", [P, H], dt.float32, isOutput=False)
    W2p = nc.declare_dram_parameter("W2", [H, C], dt.bfloat16, isOutput=False)
    b2p = nc.declare_dram_parameter("b2r", [P, C], dt.float32, isOutput=False)
    ixp = nc.declare_dram_parameter("idxp", [P, totcols * 8], dt.int16, isOutput=False)
    smp = nc.declare_dram_parameter(
        "smat", [P, totcols * P], dt.bfloat16, isOutput=False
    )
    outp = nc.declare_dram_parameter("out", [NPAD, C], dt.float32, isOutput=True)

    rg = [list(range(N_CORES))]

    calls = []  # (chunk, global col start, ncols)
    goff = 0
    for c in range(NCH):
        o = 0
        while o < ncols_ch[c]:
            n = min(CALLCOLS, ncols_ch[c] - o)
            calls.append((c, goff + o, n))
            o += n
        goff += ncols_ch[c]

    # chunks contributing per tile (always NCH here since ncol>=1 everywhere)
    tile_nch = [NCH] * NW

    with TileContext(nc) as tc:
        with (
            tc.tile_pool(name="const", bufs=1) as const,
            tc.tile_pool(name="dram", bufs=1, space="DRAM") as dram,
            tc.tile_pool(name="xp", bufs=3) as xp,
            tc.tile_pool(name="hb", bufs=2) as hb,
            tc.tile_pool(name="ixpool", bufs=4) as ixpool,
            tc.tile_pool(name="smpool", bufs=4) as smpool,
            tc.tile_pool(name="gpool", bufs=4) as gpool,
            tc.tile_pool(name="acc", bufs=1) as accp,
            tc.tile_pool(name="sp", bufs=4) as sp,
            tc.tile_pool(name="ps", bufs=2, space="PSUM") as ps,
        ):
            w1sb = const.tile([P, KF, H], dt.bfloat16)
            nc.sync.dma_start(out=w1sb[:], in_=W1p[:].rearrange("(c p) h -> p c h", p=P))
            w2sb = const.tile([H, C], dt.bfloat16)
            nc.sync.dma_start(out=w2sb[:], in_=W2p[:])
            b1sb = const.tile([P, H], dt.float32)
            nc.sync.dma_start(out=b1sb[:], in_=b1p[:])
            b2sb = const.tile([P, C], dt.float32)
            nc.sync.dma_start(out=b2sb[:], in_=b2p[:])
            ident = const.tile([P, P], dt.bfloat16)
            make_identity(nc, ident[:])

            h1k = dram.tile([NPAD, TW], dt.bfloat16)
            t2k = dram.tile([NPAD, TW], dt.bfloat16)
            tb1 = dram.tile([TOT, TW], dt.bfloat16)
            tb2 = dram.tile([TOT, TW], dt.bfloat16)

            zt = const.tile([1, TW], dt.bfloat16)
            nc.gpsimd.memset(zt[:], 0.0)
            for c in range(NCH):
                nc.sync.dma_start(out=tb1[CB[c]:CB[c] + 1, :], in_=zt[:])
                nc.sync.dma_start(out=tb2[CB[c]:CB[c] + 1, :], in_=zt[:])

            # per-tile accumulators (separate tiles -> no hazard serialization)
            acc1 = [accp.tile([P, H], dt.float32, name=f"acc1_{t}") for t in range(NW)]
            acc2 = [accp.tile([P, C], dt.float32, name=f"acc2_{t}") for t in range(NW)]

            # --- phase 1: h1 = x@W1 + b1, chunked AllGather
            xTr = xT[:].rearrange("(c p) n -> p c n", p=P)
            for c in range(NCH):
                t0, t1 = int(cfg["TSTART"][c]), int(cfg["TSTART"][c + 1])
                ntl = t1 - t0
                h1b = hb.tile([P, ntl, TW], dt.bfloat16, tag="h1b", name=f"h1b_{c}")
                nc.vector.memset(h1b[:].rearrange("p a b -> p (a b)"), 0.0)
                for i, nt in enumerate(range(t0, t1)):
                    cs_ = min(P, NP - nt * P)
                    if cs_ <= 0:
                        continue
                    xt = xp.tile([P, KF, P], dt.bfloat16, tag="xt")
                    nc.sync.dma_start(out=xt[:, :, :cs_], in_=xTr[:, :, nt * P:nt * P + cs_])
                    ph = ps.tile([P, H], dt.float32, tag="ph", bufs=2)
                    for kf in range(KF):
                        nc.tensor.matmul(
                            out=ph[:cs_, :], lhsT=xt[:, kf, :cs_], rhs=w1sb[:, kf, :],
                            start=(kf == 0), stop=(kf == KF - 1),
                        )
                    nc.vector.tensor_tensor(
                        out=h1b[:cs_, i, :H], in0=ph[:cs_, :], in1=b1sb[:cs_, :],
                        op=mybir.AluOpType.add,
                    )
                nc.sync.dma_start(
                    out=h1k[CS[c]:CS[c] + ntl * P, :].rearrange(
                        "(a p) w -> p a w", p=P),
                    in_=h1b[:],
                )
                nc.gpsimd.collective_compute(
                    "AllGather", mybir.AluOpType.bypass, replica_groups=rg,
                    ins=[h1k[CS[c]:CS[c] + CZ[c], :]],
                    outs=[tb1[CB[c] + 1:CB[c] + 1 + N_CORES * CZ[c], :]],
                )

            def agg_pass(tbl, width, acc, post_tile):
                pend = {}
                first = {}
                for (c, g0, ncols) in calls:
                    NI = ncols * P
                    ixt = ixpool.tile([P, ncols * 8], dt.int16, tag="ix")
                    nc.sync.dma_start(out=ixt[:], in_=ixp[:, g0 * 8:(g0 + ncols) * 8])
                    smt = smpool.tile([P, ncols, P], dt.bfloat16, tag="sm")
                    nc.sync.dma_start(
                        out=smt[:],
                        in_=smp[:, g0 * P:(g0 + ncols) * P].rearrange(
                            "p (n s) -> p n s", s=P),
                    )
                    gt = gpool.tile([P, ncols, TW], dt.bfloat16, tag="gt")
                    nc.gpsimd.dma_gather(
                        gt[:], tbl[CB[c]:CB[c] + CROWS[c], :], ixt[:], NI, NI, TW,
                        single_packet=False,
                    )
                    base = sum(ncols_ch[:c])
                    for j in range(ncols):
                        t, st, sp_, tc_last = meta[c][g0 - base + j]
                        if t not in pend:
                            pend[t] = ps.tile(
                                [P, width], dt.float32, tag="agg", bufs=2,
                                name=f"agg_{c}_{t}",
                            )
                        nc.tensor.matmul(
                            out=pend[t][:], lhsT=smt[:, j, :], rhs=gt[:, j, :width],
                            start=st, stop=sp_,
                        )
                        if tc_last:
                            pt = pend.pop(t)
                            if t not in first:
                                first[t] = 1
                                nc.vector.tensor_copy(out=acc[t][:], in_=pt[:])
                            else:
                                first[t] += 1
                                nc.vector.tensor_tensor(
                                    out=acc[t][:], in0=acc[t][:], in1=pt[:],
                                    op=mybir.AluOpType.add,
                                )
                            if first[t] == tile_nch[t]:
                                post_tile(t)

            # --- phase 2: L1 aggregate -> elu' -> t2 rows -> AllGather#2
            t2bufs = {}
            t2done = {}

            def make_t2(t):
                c = int(np.searchsorted(cfg["TSTART"], t, side="right") - 1)
                t0, t1 = int(cfg["TSTART"][c]), int(cfg["TSTART"][c + 1])
                if c not in t2bufs:
                    t2bufs[c] = hb.tile(
                        [P, t1 - t0, TW], dt.bfloat16, tag="t2b", name=f"t2b_{c}",
                    )
                    t2done[c] = 0
                    nc.vector.memset(t2bufs[c][:].rearrange("p a b -> p (a b)"), 0.0)
                red = acc1[t][:]
                m = sp.tile([P, H], dt.float32, tag="m")
                nc.vector.tensor_scalar_min(out=m[:], in0=red, scalar1=0.0)
                e = sp.tile([P, H], dt.float32, tag="e")
                nc.scalar.activation(e[:], m[:], mybir.ActivationFunctionType.Exp)
                gpr = sp.tile([P, H], dt.bfloat16, tag="gpr")
                nc.vector.scalar_tensor_tensor(
                    out=gpr[:], in0=red, scalar=0.0, in1=e[:],
                    op0=mybir.AluOpType.max, op1=mybir.AluOpType.add,
                )
                tr = ps.tile([H, P], dt.bfloat16, tag="tr", bufs=2)
                nc.tensor.transpose(out=tr[:], in_=gpr[:], identity=ident[:])
                trsb = sp.tile([H, P], dt.bfloat16, tag="trsb")
                nc.vector.tensor_copy(out=trsb[:], in_=tr[:])
                t2p = ps.tile([P, C], dt.float32, tag="t2p", bufs=2)
                nc.tensor.matmul(out=t2p[:], lhsT=trsb[:], rhs=w2sb[:, :C],
                                 start=True, stop=True)
                nc.vector.tensor_tensor(
                    out=t2bufs[c][:, t - t0, :C], in0=t2p[:], in1=b2sb[:, :C],
                    op=mybir.AluOpType.add,
                )
                t2done[c] += 1
                if t2done[c] == t1 - t0:
                    nc.sync.dma_start(
                        out=t2k[CS[c]:CS[c] + (t1 - t0) * P, :].rearrange(
                            "(a p) w -> p a w", p=P),
                        in_=t2bufs[c][:],
                    )
                    nc.gpsimd.collective_compute(
                        "AllGather", mybir.AluOpType.bypass, replica_groups=rg,
                        ins=[t2k[CS[c]:CS[c] + CZ[c], :]],
                        outs=[tb2[CB[c] + 1:CB[c] + 1 + N_CORES * CZ[c], :]],
                    )

            agg_pass(tb1, H, acc1, make_t2)

            # --- phase 3: L2 aggregate -> log_softmax (all per-tile tiles)
            def softmax_t(t):
                red = acc2[t][:]
                nm = sp.tile([P, 1], dt.float32, tag="nm")
                nc.vector.tensor_reduce(
                    out=nm[:], in_=red, axis=mybir.AxisListType.X,
                    op=mybir.AluOpType.max, negate=True,
                )
                sc = sp.tile([P, C], dt.float32, tag="sc")
                ssum = sp.tile([P, 1], dt.float32, tag="ssum")
                nc.scalar.activation(
                    sc[:], red, mybir.ActivationFunctionType.Exp,
                    bias=nm[:], accum_out=ssum[:],
                )
                ls = sp.tile([P, 1], dt.float32, tag="ls")
                nc.scalar.activation(ls[:], ssum[:], mybir.ActivationFunctionType.Ln)
                nc.vector.tensor_scalar(
                    out=acc2[t][:], in0=red, scalar1=nm[:], scalar2=ls[:],
                    op0=mybir.AluOpType.add, op1=mybir.AluOpType.subtract,
                )
                nc.sync.dma_start(
                    out=outp[t * P:(t + 1) * P, :], in_=acc2[t][:],
                )

            agg_pass(tb2, C, acc2, softmax_t)

    nc.compile()
    return nc


# ---------------------------------------------------------------- entry point

LAST_RESULT = {}


def _run(cfg, x, edge_index, W1, b1, W2, b2, trace=False):
    from concourse.bass_utils import run_bass_kernel_spmd

    sched, in_maps = host_prep(cfg, x, edge_index, W1, b1, W2, b2)
    nc = build_program(cfg, sched)
    res = run_bass_kernel_spmd(
        nc, in_maps, list(range(N_CORES)), trace=trace,
    )
    LAST_RESULT["exec_time_ns"] = res.exec_time_ns
    LAST_RESULT["mean_exec_time_ns"] = res.mean_exec_time_ns
    N, NP, C = cfg["N"], cfg["NP"], cfg["C"]
    full = np.empty((N, C), dtype=np.float32)
    for k in range(N_CORES):
        outk = np.asarray(res.results[k]["out"], dtype=np.float32)
        full[k * NP:(k + 1) * NP] = outk[:NP]
    return full


def kernel(x, edge_index, W1, b1, W2, b2):
    trace = bool(int(os.environ.get("GNN_TRACE", "0")))
    return _run(FULL_CFG, x, edge_index, W1, b1, W2, b2, trace=trace)


# revision 21
# speedup vs baseline: 1.0114x; 1.0114x over previous
"""GNN message-passing v4: batched dma_gather + tile-level selection matmuls.

vs v2: SEG=128 (one selection matrix per column covering the whole 128-dst
tile -> minimal padding), per-tile accumulator tiles (no shared-tile hazard
serialization), gather ring depth 3.
"""

import os
import sys

sys.path.insert(0, "/opt/trn_rl_repo")

import numpy as np
import ml_dtypes

BF16 = ml_dtypes.bfloat16

N_CORES = 8
P = 128
CALLCOLS = 64     # 128-edge columns per dma_gather call (num_idxs 8192, HW-validated)
TW = 128          # table row width (bf16 elements) -> 256B rows


def _make_cfg(n_nodes, n_edges, f_in=512, hid=64, n_cls=40):
    np_ = n_nodes // N_CORES
    assert np_ * N_CORES == n_nodes
    nw = (np_ + P - 1) // P
    npad = nw * P
    n_chunks = min(4, nw)
    tiles = [nw // n_chunks + (1 if i < nw % n_chunks else 0) for i in range(n_chunks)]
    tstart = np.concatenate([[0], np.cumsum(tiles)]).astype(int)
    cs = [int(tstart[i] * P) for i in range(n_chunks + 1)]
    cz = [cs[i + 1] - cs[i] for i in range(n_chunks)]
    crows = [1 + N_CORES * z for z in cz]
    cb = np.concatenate([[0], np.cumsum(crows)]).astype(int)
    assert max(crows) <= 32767
    return dict(
        N=n_nodes, E=n_edges, F=f_in, H=hid, C=n_cls, NP=np_, NW=nw, NPAD=npad,
        NCH=n_chunks, TILES=tiles, TSTART=tstart, CS=cs, CZ=cz,
        CROWS=crows, CB=cb, TOT=int(cb[-1]),
    )


FULL_CFG = _make_cfg(100000, 3200000)


# ---------------------------------------------------------------- host prep

def host_prep(cfg, x, edge_index, W1, b1, W2, b2):
    N, NP, NW, NCH = cfg["N"], cfg["NP"], cfg["NW"], cfg["NCH"]
    CS, CZ, CB = cfg["CS"], cfg["CZ"], cfg["CB"]
    src = np.asarray(edge_index[0]).astype(np.int64)
    dst = np.asarray(edge_index[1]).astype(np.int64)

    r = np.arange(N, dtype=np.int64) // NP
    l = np.arange(N, dtype=np.int64) % NP
    csb = np.asarray(CS)
    ch_of = np.searchsorted(csb, l, side="right") - 1
    czv = np.asarray(CZ + [1])[ch_of]
    locrow = 1 + r * czv + (l - csb[ch_of])

    # per-core sorted edge views + per-(chunk, tile) counts
    views = []
    cnts = np.zeros((N_CORES, NCH, NW), dtype=np.int64)
    for k in range(N_CORES):
        sel = (dst >= k * NP) & (dst < (k + 1) * NP)
        s_k = src[sel]
        d_k = dst[sel] - k * NP
        key = ch_of[s_k] * (NP + 1) + d_k
        order = np.argsort(key, kind="stable")
        s_k, d_k = s_k[order], d_k[order]
        ch_k = ch_of[s_k]
        lr_k = locrow[s_k]
        views.append((d_k, ch_k, lr_k))
        for c in range(NCH):
            m = ch_k == c
            tt = d_k[m] // P
            bc = np.bincount(tt, minlength=NW)
            cnts[k, c, :] = bc

    # common column layout: per (chunk, tile) max over cores of ceil(cnt/128)
    ncol_ct = np.maximum(1, (cnts + P - 1) // P).max(axis=0)  # [NCH, NW]
    # meta per chunk: list of (tile, start, stop, tc_last)
    common_meta = []
    for c in range(NCH):
        meta_c = []
        for t in range(NW):
            nc_ = int(ncol_ct[c, t])
            for j in range(nc_):
                meta_c.append([t, j == 0, j == nc_ - 1, j == nc_ - 1])
        common_meta.append(meta_c)
    totcols = int(ncol_ct.sum())

    # per-core index streams + selection matrices on the common layout
    idxp_all, s_all = [], []
    for k in range(N_CORES):
        d_k, ch_k, lr_k = views[k]
        idx_list, s_list = [], []
        for c in range(NCH):
            m = ch_k == c
            dc = d_k[m]
            lrc = lr_k[m]
            tt = dc // P
            # edges already sorted by dst within chunk -> tiles contiguous
            tstarts = np.concatenate([[0], np.cumsum(np.bincount(tt, minlength=NW))])
            for t in range(NW):
                a, b = int(tstarts[t]), int(tstarts[t + 1])
                cnt = b - a
                nc_ = int(ncol_ct[c, t])
                rows = np.zeros(nc_ * P, dtype=np.int16)
                rows[:cnt] = lrc[a:b]
                dloc = np.full(nc_ * P, -1, dtype=np.int64)
                dloc[:cnt] = dc[a:b] - t * P
                for j in range(nc_):
                    sm = np.zeros((P, P), dtype=BF16)
                    dj = dloc[j * P:(j + 1) * P]
                    val = dj >= 0
                    sm[np.arange(P)[val], dj[val]] = 1.0
                    s_list.append(sm)
                    idx_list.append(rows[j * P:(j + 1) * P])
        st = np.concatenate(idx_list)
        t16 = st.reshape(totcols * 8, 16).T
        idxp_all.append(np.ascontiguousarray(np.tile(t16, (8, 1))))
        s_all.append(np.ascontiguousarray(
            np.stack(s_list, axis=1).reshape(P, -1)))

    W1b = np.asarray(W1, dtype=np.float32).astype(BF16)
    W2b = np.asarray(W2, dtype=np.float32).astype(BF16)
    b1r = np.tile(np.asarray(b1, dtype=np.float32)[None, :], (P, 1))
    b2a = np.asarray(b2, dtype=np.float32) - np.asarray(W2, np.float32).sum(0)
    b2r = np.tile(b2a[None, :], (P, 1))
    in_maps = []
    xf = np.asarray(x, dtype=np.float32)
    for k in range(N_CORES):
        xT = np.ascontiguousarray(xf[k * NP:(k + 1) * NP].T).astype(BF16)
        in_maps.append(dict(
            xT=xT, W1=W1b, b1r=b1r, W2=W2b, b2r=b2r,
            idxp=idxp_all[k], smat=s_all[k],
        ))
    sched = dict(common_meta=common_meta, totcols=totcols)
    return sched, in_maps


# ---------------------------------------------------------------- device code

def build_program(cfg, sched):
    import concourse.bass as bass
    import concourse.bacc as bacc
    import concourse.mybir as mybir
    from concourse.tile import TileContext
    from concourse.masks import make_identity

    dt = mybir.dt
    N, F, H, C = cfg["N"], cfg["F"], cfg["H"], cfg["C"]
    NP, NW, NPAD, NCH = cfg["NP"], cfg["NW"], cfg["NPAD"], cfg["NCH"]
    CS, CZ, CB, CROWS = cfg["CS"], cfg["CZ"], cfg["CB"], cfg["CROWS"]
    TOT = cfg["TOT"]
    meta = sched["common_meta"]
    ncols_ch = [len(m) for m in meta]
    totcols = sched["totcols"]
    KF = F // P

    nc = bacc.Bacc(
        "TRN2", target_bir_lowering=False, debug=False, num_devices=N_CORES
    )
    xT = nc.declare_dram_parameter("xT", [F, NP], dt.bfloat16, isOutput=False)
    W1p = nc.declare_dram_parameter("W1", [F, H], dt.bfloat16, isOutput=False)
    b1p = nc.declare_dram_parameter("b1r", [P, H], dt.float32, isOutput=False)
    W2p = nc.declare_dram_parameter("W2", [H, C], dt.bfloat16, isOutput=False)
    b2p = nc.declare_dram_parameter("b2r", [P, C], dt.float32, isOutput=False)
    ixp = nc.declare_dram_parameter("idxp", [P, totcols * 8], dt.int16, isOutput=False)
    smp = nc.declare_dram_parameter(
        "smat", [P, totcols * P], dt.bfloat16, isOutput=False
    )
    outp = nc.declare_dram_parameter("out", [NPAD, C], dt.float32, isOutput=True)

    rg = [list(range(N_CORES))]

    calls = []  # (chunk, global col start, ncols)
    goff = 0
    for c in range(NCH):
        o = 0
        while o < ncols_ch[c]:
            n = min(CALLCOLS, ncols_ch[c] - o)
            calls.append((c, goff + o, n))
            o += n
        goff += ncols_ch[c]

    # chunks contributing per tile (always NCH here since ncol>=1 everywhere)
    tile_nch = [NCH] * NW

    with TileContext(nc) as tc:
        with (
            tc.tile_pool(name="const", bufs=1) as const,
            tc.tile_pool(name="dram", bufs=1, space="DRAM") as dram,
            tc.tile_pool(name="xp", bufs=3) as xp,
            tc.tile_pool(name="hb", bufs=2) as hb,
            tc.tile_pool(name="ixpool", bufs=3) as ixpool,
            tc.tile_pool(name="smpool", bufs=3) as smpool,
            tc.tile_pool(name="gpool", bufs=3) as gpool,
            tc.tile_pool(name="acc", bufs=1) as accp,
            tc.tile_pool(name="sp", bufs=4) as sp,
            tc.tile_pool(name="ps", bufs=2, space="PSUM") as ps,
        ):
            w1sb = const.tile([P, KF, H], dt.bfloat16)
            nc.sync.dma_start(out=w1sb[:], in_=W1p[:].rearrange("(c p) h -> p c h", p=P))
            w2sb = const.tile([H, C], dt.bfloat16)
            nc.sync.dma_start(out=w2sb[:], in_=W2p[:])
            b1sb = const.tile([P, H], dt.float32)
            nc.sync.dma_start(out=b1sb[:], in_=b1p[:])
            b2sb = const.tile([P, C], dt.float32)
            nc.sync.dma_start(out=b2sb[:], in_=b2p[:])
            ident = const.tile([P, P], dt.bfloat16)
            make_identity(nc, ident[:])

            h1k = dram.tile([NPAD, TW], dt.bfloat16)
            t2k = dram.tile([NPAD, TW], dt.bfloat16)
            tb1 = dram.tile([TOT, TW], dt.bfloat16)
            tb2 = dram.tile([TOT, TW], dt.bfloat16)

            zt = const.tile([1, TW], dt.bfloat16)
            nc.gpsimd.memset(zt[:], 0.0)
            for c in range(NCH):
                nc.sync.dma_start(out=tb1[CB[c]:CB[c] + 1, :], in_=zt[:])
                nc.sync.dma_start(out=tb2[CB[c]:CB[c] + 1, :], in_=zt[:])

            # per-tile accumulators (separate tiles -> no hazard serialization)
            acc1 = [accp.tile([P, H], dt.float32, name=f"acc1_{t}") for t in range(NW)]
            acc2 = [accp.tile([P, C], dt.float32, name=f"acc2_{t}") for t in range(NW)]

            # --- phase 1: h1 = x@W1 + b1, chunked AllGather
            xTr = xT[:].rearrange("(c p) n -> p c n", p=P)
            for c in range(NCH):
                t0, t1 = int(cfg["TSTART"][c]), int(cfg["TSTART"][c + 1])
                ntl = t1 - t0
                h1b = hb.tile([P, ntl, TW], dt.bfloat16, tag="h1b", name=f"h1b_{c}")
                nc.vector.memset(h1b[:].rearrange("p a b -> p (a b)"), 0.0)
                for i, nt in enumerate(range(t0, t1)):
                    cs_ = min(P, NP - nt * P)
                    if cs_ <= 0:
                        continue
                    xt = xp.tile([P, KF, P], dt.bfloat16, tag="xt")
                    nc.sync.dma_start(out=xt[:, :, :cs_], in_=xTr[:, :, nt * P:nt * P + cs_])
                    ph = ps.tile([P, H], dt.float32, tag="ph", bufs=2)
                    for kf in range(KF):
                        nc.tensor.matmul(
                            out=ph[:cs_, :], lhsT=xt[:, kf, :cs_], rhs=w1sb[:, kf, :],
                            start=(kf == 0), stop=(kf == KF - 1),
                        )
                    nc.vector.tensor_tensor(
                        out=h1b[:cs_, i, :H], in0=ph[:cs_, :], in1=b1sb[:cs_, :],
                        op=mybir.AluOpType.add,
                    )
                nc.sync.dma_start(
                    out=h1k[CS[c]:CS[c] + ntl * P, :].rearrange(
                        "(a p) w -> p a w", p=P),
                    in_=h1b[:],
                )
                nc.gpsimd.collective_compute(
                    "AllGather", mybir.AluOpType.bypass, replica_groups=rg,
                    ins=[h1k[CS[c]:CS[c] + CZ[c], :]],
                    outs=[tb1[CB[c] + 1:CB[c] + 1 + N_CORES * CZ[c], :]],
                )

            def agg_pass(tbl, width, acc, post_tile):
                pend = {}
                first = {}
                for (c, g0, ncols) in calls:
                    NI = ncols * P
                    ixt = ixpool.tile([P, ncols * 8], dt.int16, tag="ix")
                    nc.sync.dma_start(out=ixt[:], in_=ixp[:, g0 * 8:(g0 + ncols) * 8])
                    smt = smpool.tile([P, ncols, P], dt.bfloat16, tag="sm")
                    nc.sync.dma_start(
                        out=smt[:],
                        in_=smp[:, g0 * P:(g0 + ncols) * P].rearrange(
                            "p (n s) -> p n s", s=P),
                    )
                    gt = gpool.tile([P, ncols, TW], dt.bfloat16, tag="gt")
                    nc.gpsimd.dma_gather(
                        gt[:], tbl[CB[c]:CB[c] + CROWS[c], :], ixt[:], NI, NI, TW,
                        single_packet=False,
                    )
                    base = sum(ncols_ch[:c])
                    for j in range(ncols):
                        t, st, sp_, tc_last = meta[c][g0 - base + j]
                        if t not in pend:
                            pend[t] = ps.tile(
                                [P, width], dt.float32, tag="agg", bufs=2,
                                name=f"agg_{c}_{t}",
                            )
                        nc.tensor.matmul(
                            out=pend[t][:], lhsT=smt[:, j, :], rhs=gt[:, j, :width],
                            start=st, stop=sp_,
                        )
                        if tc_last:
                            pt = pend.pop(t)
                            if t not in first:
                                first[t] = 1
                                nc.vector.tensor_copy(out=acc[t][:], in_=pt[:])
                            else:
                                first[t] += 1
                                nc.vector.tensor_tensor(
                                    out=acc[t][:], in0=acc[t][:], in1=pt[:],
                                    op=mybir.AluOpType.add,
                                )
                            if first[t] == tile_nch[t]:
                                post_tile(t)

            # --- phase 2: L1 aggregate -> elu' -> t2 rows -> AllGather#2
            t2bufs = {}
            t2done = {}

            def make_t2(t):
                c = int(np.searchsorted(cfg["TSTART"], t, side="right") - 1)
                t0, t1 = int(cfg["TSTART"][c]), int(cfg["TSTART"][c + 1])
                if c not in t2bufs:
                    t2bufs[c] = hb.tile(
                        [P, t1 - t0, TW], dt.bfloat16, tag="t2b", name=f"t2b_{c}",
                    )
                    t2done[c] = 0
                    nc.vector.memset(t2bufs[c][:].rearrange("p a b -> p (a b)"), 0.0)
                red = acc1[t][:]
                m = sp.tile([P, H], dt.float32, tag="m")
                nc.vector.tensor_scalar_min(out=m[:], in0=red, scalar1=0.0)
                e = sp.tile([P, H], dt.float32, tag="e")
                nc.scalar.activation(e[:], m[:], mybir.ActivationFunctionType.Exp)
                gpr = sp.tile([P, H], dt.bfloat16, tag="gpr")
                nc.vector.scalar_tensor_tensor(
                    out=gpr[:], in0=red, scalar=0.0, in1=e[:],
                    op0=mybir.AluOpType.max, op1=mybir.AluOpType.add,
                )
                tr = ps.tile([H, P], dt.bfloat16, tag="tr", bufs=2)
                nc.tensor.transpose(out=tr[:], in_=gpr[:], identity=ident[:])
                trsb = sp.tile([H, P], dt.bfloat16, tag="trsb")
                nc.vector.tensor_copy(out=trsb[:], in_=tr[:])
                t2p = ps.tile([P, C], dt.float32, tag="t2p", bufs=2)
                nc.tensor.matmul(out=t2p[:], lhsT=trsb[:], rhs=w2sb[:, :C],
                                 start=True, stop=True)
                nc.vector.tensor_tensor(
                    out=t2bufs[c][:, t - t0, :C], in0=t2p[:], in1=b2sb[:, :C],
                    op=mybir.AluOpType.add,
                )
                t2done[c] += 1
                if t2done[c] == t1 - t0:
                    nc.sync.dma_start(
                        out=t2k[CS[c]:CS[c] + (t1 - t0) * P, :].rearrange(
                            "(a p) w -> p a w", p=P),
                        in_=t2bufs[c][:],
                    )
                    nc.gpsimd.collective_compute(
                        "AllGather", mybir.AluOpType.bypass, replica_groups=rg,
                        ins=[t2k[CS[c]:CS[c] + CZ[c], :]],
                        outs=[tb2[CB[c] + 1:CB[c] + 1 + N_CORES * CZ[c], :]],
                    )

            agg_pass(tb1, H, acc1, make_t2)

            # --- phase 3: L2 aggregate -> log_softmax (all per-tile tiles)
            def softmax_t(t):
                red = acc2[t][:]
                nm = sp.tile([P, 1], dt.float32, tag="nm")
                nc.vector.tensor_reduce(
                    out=nm[:], in_=red, axis=mybir.AxisListType.X,
                    op=mybir.AluOpType.max, negate=True,
                )
                sc = sp.tile([P, C], dt.float32, tag="sc")
                ssum = sp.tile([P, 1], dt.float32, tag="ssum")
                nc.scalar.activation(
                    sc[:], red, mybir.ActivationFunctionType.Exp,
                    bias=nm[:], accum_out=ssum[:],
                )
                ls = sp.tile([P, 1], dt.float32, tag="ls")
                nc.scalar.activation(ls[:], ssum[:], mybir.ActivationFunctionType.Ln)
                nc.vector.tensor_scalar(
                    out=acc2[t][:], in0=red, scalar1=nm[:], scalar2=ls[:],
                    op0=mybir.AluOpType.add, op1=mybir.AluOpType.subtract,
                )
                nc.sync.dma_start(
                    out=outp[t * P:(t + 1) * P, :], in_=acc2[t][:],
                )

            agg_pass(tb2, C, acc2, softmax_t)

    nc.compile()
    return nc


# ---------------------------------------------------------------- entry point

LAST_RESULT = {}


def _run(cfg, x, edge_index, W1, b1, W2, b2, trace=False):
    from concourse.bass_utils import run_bass_kernel_spmd

    sched, in_maps = host_prep(cfg, x, edge_index, W1, b1, W2, b2)
    nc = build_program(cfg, sched)
    res = run_bass_kernel_spmd(
        nc, in_maps, list(range(N_CORES)), trace=trace,
    )
    LAST_RESULT["exec_time_ns"] = res.exec_time_ns
    LAST_RESULT["mean_exec_time_ns"] = res.mean_exec_time_ns
    N, NP, C = cfg["N"], cfg["NP"], cfg["C"]
    full = np.empty((N, C), dtype=np.float32)
    for k in range(N_CORES):
        outk = np.asarray(res.results[k]["out"], dtype=np.float32)
        full[k * NP:(k + 1) * NP] = outk[:NP]
    return full


def kernel(x, edge_index, W1, b1, W2, b2):
    trace = bool(int(os.environ.get("GNN_TRACE", "0")))
    return _run(FULL_CFG, x, edge_index, W1, b1, W2, b2, trace=trace)
